# revision 29
# baseline (speedup 1.0000x reference)
"""Trainium2 8-core kernel for nn_CausalSelfAttention_11192684774089.

Computation (see reference): qkv = x@W_attn + b; LoRA on q,k; RoPE on q,k;
causal softmax attention; out = y@W_proj + b_proj.
  B=4, T=2048, C=2048, H=16 heads, D=128, fp32 I/O.

Sharding: tensor-parallel over heads (2 heads/core) for QKV + attention,
then an AllToAll switches to row-parallel for the output projection
(Megatron sequence-parallel style: A2A moves 4.2MB/core instead of a
67MB AllReduce). Host concatenates the 8 row-shards.

Device algorithm (per core):
  - LoRA is folded into effective weights on host: W_eff = W(I + s A B).
  - x is passed pre-transposed+bf16 [C, B*T]; QKV computed channel-major
    (q^T,k^T [128chan, ROWS]) and row-major for v, all SBUF-resident.
  - RoPE applied channel-major via a partition-permute DMA + 3 DVE ops,
    with host-precomputed cos/sin tables [128, T].
  - Attention in S^T layout: scores^T[key,q] tiles via PE, exp on ACT
    (fused scale 1/sqrt(D)), causal mask via host-precomputed 0/1 table,
    denominators via ones-vector matmul, AV^T via PE, division by the
    (partition-broadcast) reciprocal on DVE.
  - AllToAll (bf16) -> y^T full-channel row-block; proj row-major + bias.
"""

import os
from contextlib import ExitStack

import numpy as np
import ml_dtypes

import concourse.bass as bass
import concourse.mybir as mybir
import concourse.tile as tile
from concourse import bacc
from concourse.masks import make_identity
from concourse.bass_utils import run_bass_kernel_spmd

# This image's antenv lacks axon_hooks; run_bass_kernel_spmd(trace=True)
# imports it unconditionally. Register a working ctypes-based NTFF hook so
# tracing works (and doesn't crash) regardless of BASS_TRACE.
try:
    import antenv.axon_hooks  # noqa: F401
except ImportError:
    import sys as _sys
    import types as _types
    _hooks = _types.ModuleType("antenv.axon_hooks")
    try:
        from trn_agent_boot.trn_boot import _ntff_profile_via_ctypes
        _HOOK = _ntff_profile_via_ctypes("/opt/axon/libaxon_pjrt.so")
    except Exception:
        _HOOK = None
    _hooks.get_axon_ntff_profile_hook = lambda: _HOOK
    _hooks.set_axon_ntff_profile_hook = lambda h: None
    _sys.modules["antenv.axon_hooks"] = _hooks
    import concourse.bass_utils as _bu
    _orig_upload = _bu.upload_artifacts

    def _safe_upload(tmpdir):
        try:
            return _orig_upload(tmpdir)
        except Exception:
            return tmpdir

    _bu.upload_artifacts = _safe_upload

BF16 = ml_dtypes.bfloat16
FP32 = np.float32

# ---------------------------------------------------------------- config


class Cfg:
    def __init__(self, B=4, T=2048, C=2048, H=16, NC=8):
        self.B, self.T, self.C, self.H, self.NC = B, T, C, H, NC
        self.D = 128                      # head dim (fixed: RoPE tables assume 128)
        assert C == H * self.D
        self.H_LOC = H // NC              # heads per core
        self.OCQ = self.H_LOC * 128       # q chans per core
        self.OC = 3 * self.OCQ            # qkv chans per core
        self.ROWS = B * T
        self.RPC = self.ROWS // NC        # output rows per core
        self.KC = C // 128                # contraction chunks
        self.RT = 512                     # qkv row tile
        self.QT = 512                     # attention q tile
        self.KT = 128                     # attention key tile
        assert T % self.QT == 0 and self.RPC % 128 == 0
        assert self.QT % self.KT == 0 and self.ROWS % self.RT == 0
        self.SCALE = 1.0 / float(np.sqrt(self.D))
        # row-slice split for pipelined A2A+proj (needs 128-divisible halves)
        self.NSPL = 2 if (self.RPC // 2) % 128 == 0 else 1
        self.SPL = self.RPC // self.NSPL


CFG = Cfg()

# ---------------------------------------------------------------- builder


def build(cfg: Cfg, debug: bool = False):
    bf = mybir.dt.bfloat16
    f32 = mybir.dt.float32
    nc = bacc.Bacc(None, debug=debug, num_devices=cfg.NC)

    B, T, C, NC = cfg.B, cfg.T, cfg.C, cfg.NC
    H_LOC, OCQ, OC = cfg.H_LOC, cfg.OCQ, cfg.OC
    ROWS, RPC, KC, RT, QT, KT = cfg.ROWS, cfg.RPC, cfg.KC, cfg.RT, cfg.QT, cfg.KT
    NQK = 2 * H_LOC                       # number of q+k 128-chan blocks
    VOC = OCQ                             # v chans per core
    RB = ROWS // 128                      # v row blocks
    Copy = mybir.ActivationFunctionType.Copy
    Ident = mybir.ActivationFunctionType.Identity
    Exp = mybir.ActivationFunctionType.Exp

    xT = nc.declare_dram_parameter("xT", [C, ROWS], bf, isOutput=False)
    w_eff = nc.declare_dram_parameter("w_eff", [C, OC], bf, isOutput=False)
    b_qk = nc.declare_dram_parameter("b_qk", [128, NQK], f32, isOutput=False)
    b_v = nc.declare_dram_parameter("b_v", [128, VOC], f32, isOutput=False)
    w_proj = nc.declare_dram_parameter("w_proj", [C, C], bf, isOutput=False)
    b_proj = nc.declare_dram_parameter("b_proj", [128, C], f32, isOutput=False)
    cosT = nc.declare_dram_parameter("cosT", [128, T], bf, isOutput=False)
    sinTs = nc.declare_dram_parameter("sinTs", [128, T], bf, isOutput=False)
    mask0 = nc.declare_dram_parameter("mask0", [128, 384 + QT], bf, isOutput=False)
    out_ext = nc.declare_dram_parameter("out", [RPC, C], f32, isOutput=True)

    with tile.TileContext(nc) as tc, ExitStack() as top:
        const = top.enter_context(tc.tile_pool(name="const", bufs=1))
        dram = top.enter_context(tc.tile_pool(name="dram", bufs=1, space="DRAM"))

        # ---- constants in SBUF
        cos_sb = const.tile([128, T], bf)
        sin_sb = const.tile([128, T], bf)
        m0_sb = const.tile([128, 384 + QT], bf)
        bqk_sb = const.tile([128, NQK], f32)
        bv_sb = const.tile([128, VOC], f32)
        bproj_sb = const.tile([128, C], f32)
        ones_sb = const.tile([128, 1], bf)
        ones1f = const.tile([1, 128], f32)
        ident_sb = const.tile([128, 128], bf)
        nc.sync.dma_start(bqk_sb[:], b_qk[:, :])
        nc.vector.memset(ones_sb[:], 1.0)
        nc.vector.memset(ones1f[:], 1.0)
        make_identity(nc, ident_sb[:])
        warm_sb = const.tile([128, 128], bf)
        nc.vector.memset(warm_sb[:], 0.5)

        NSPL, SPL = cfg.NSPL, cfg.SPL
        a2a_in = [[dram.tile([NC, 128, SPL], bf, name=f"a2a_in_{h}_{s}")
                   for s in range(NSPL)] for h in range(H_LOC)]
        a2a_out = [[dram.tile([NC, 128, SPL], bf, name=f"a2a_out_{h}_{s}")
                    for s in range(NSPL)] for h in range(H_LOC)]
        qk_dram = dram.tile([128, 2 * H_LOC, ROWS], bf)

        # ---- persistent activation tiles (live into attention phase)
        act_pool = top.enter_context(tc.tile_pool(name="acts", bufs=1))
        qk_raw = act_pool.tile([128, NQK, ROWS], bf)     # q then k, chan-major
        v_sb = act_pool.tile([128, RB, VOC], bf)         # v row-major

        # ========= Phase 1: QKV + fused RoPE (per row tile) =========
        qkd = qk_dram.rearrange("(hh two) o r -> two hh o r", two=2)
        with tc.tile_pool(name="qkv_w", bufs=1) as wpool, \
             tc.tile_pool(name="qkv_x", bufs=2) as xpool, \
             tc.tile_pool(name="rope_tmp", bufs=3) as tpool, \
             tc.tile_pool(name="qkv_ps", bufs=3, space="PSUM") as qps, \
             tc.tile_pool(name="qkv_psv", bufs=2, space="PSUM") as vps:
            w_sb = wpool.tile([128, KC, OC], bf)
            w_view = w_eff.rearrange("(kc p) oc -> p kc oc", p=128)
            for rt in range(ROWS // RT):
                rsl = slice(rt * RT, (rt + 1) * RT)
                tsl = slice((rt * RT) % T, (rt * RT) % T + RT)  # t within batch
                xt = xpool.tile([128, KC, RT], bf, name="xt")
                xt_view = xT[:, rsl].rearrange("(kc p) r -> p kc r", p=128)
                if rt == 0:   # chunked first tile so MMs can start early
                    for k in range(KC):
                        nc.sync.dma_start(xt[:, k, :], xt_view[:, k, :])
                        nc.sync.dma_start(w_sb[:, k, :], w_view[:, k, :])
                    # big constants can land later; emit after the hot path
                    nc.sync.dma_start(cos_sb[:], cosT[:, :])
                    nc.sync.dma_start(sin_sb[:], sinTs[:, :])
                    nc.sync.dma_start(m0_sb[:], mask0[:, :])
                    nc.sync.dma_start(bv_sb[:], b_v[:, :])
                    nc.sync.dma_start(bproj_sb[:], b_proj[:, :])
                    # HAM warm-up: ~6us of tiny matmuls while DMAs stream in
                    wps = qps.tile([1, 128], f32, name="warm_ps")
                    for _ in range(100):
                        nc.tensor.matmul(wps[:], lhsT=ones_sb[:],
                                         rhs=warm_sb[:], start=True, stop=True)
                else:
                    nc.sync.dma_start(xt[:], xt_view)
                for o in range(NQK):
                    ps = qps.tile([128, RT], f32, name="qk_ps")
                    for k in range(KC):
                        nc.tensor.matmul(
                            ps[:], lhsT=w_sb[:, k, o * 128:(o + 1) * 128],
                            rhs=xt[:, k, :], start=(k == 0), stop=(k == KC - 1))
                    nc.scalar.activation(
                        qk_raw[:, o, rsl], ps[:], Ident, bias=bqk_sb[:, o:o + 1])
                    # RoPE, fused at row-tile granularity:
                    nc.sync.dma_start(qk_dram[:, o, rsl], qk_raw[:, o, rsl])
                    tld = tpool.tile([128, RT], bf, name="tld")
                    nc.sync.dma_start(tld[0:64, :], qkd[1, :, o, rsl])
                    nc.sync.dma_start(tld[64:128, :], qkd[0, :, o, rsl])
                    nc.vector.tensor_mul(tld[:], tld[:], sin_sb[:, tsl])
                    nc.vector.tensor_mul(
                        qk_raw[:, o, rsl], qk_raw[:, o, rsl], cos_sb[:, tsl])
                    nc.vector.tensor_add(
                        qk_raw[:, o, rsl], qk_raw[:, o, rsl], tld[:])
                for rs in range(RT // 128):
                    psv = vps.tile([128, VOC], f32, name="v_ps")
                    for k in range(KC):
                        nc.tensor.matmul(
                            psv[:], lhsT=xt[:, k, rs * 128:(rs + 1) * 128],
                            rhs=w_sb[:, k, NQK * 128:], start=(k == 0),
                            stop=(k == KC - 1))
                    nc.vector.tensor_add(
                        v_sb[:, rt * (RT // 128) + rs, :], psv[:], bv_sb[:])

        # proj weights: half loads early so the DMA overlaps attention
        OT = 512
        CH = max(C // 2, OT)
        pwpool = top.enter_context(tc.tile_pool(name="proj_w", bufs=1))
        pw_sb = pwpool.tile([128, KC, CH], bf, name="pw_sb")
        nc.sync.dma_start(
            pw_sb[:], w_proj[:, 0:CH].rearrange("(kc p) oc -> p kc oc", p=128))
        pw_halves = [pw_sb]

        # ================= Phase 3: attention + split A2A =================
        # Group q-tiles by the destination row-slice so each (head, slice)
        # AllToAll can be issued as soon as its rows are done.
        if QT <= SPL:
            groups = [[(b, qt) for b in range(B) for qt in range(T // QT)
                       if ((b * T + qt * QT) % RPC) // SPL == s]
                      for s in range(NSPL)]
        else:
            groups = [[(b, qt) for b in range(B) for qt in range(T // QT)]]
            groups += [[] for _ in range(NSPL - 1)]

        with tc.tile_pool(name="pT", bufs=2) as ppool, \
             tc.tile_pool(name="att_tmp", bufs=2) as atmp, \
             tc.tile_pool(name="rdram", bufs=4, space="DRAM") as rdram, \
             tc.tile_pool(name="s_ps", bufs=4, space="PSUM") as sps, \
             tc.tile_pool(name="d_ps", bufs=2, space="PSUM") as dps, \
             tc.tile_pool(name="av_ps", bufs=2, space="PSUM") as avps:
            for h in range(H_LOC):
                qh = qk_raw[:, h, :]
                kh = qk_raw[:, H_LOC + h, :]
                for s in range(NSPL):
                    for (b, qt) in groups[s]:
                        nk = (qt + 1) * QT // KT
                        q0 = b * T + qt * QT
                        pT = ppool.tile([128, T // KT, QT], bf, name="pT")
                        dn = dps.tile([1, QT], f32, name="dn")
                        av = avps.tile([128, QT], f32, name="av")
                        def dn_av(kk):
                            nc.tensor.matmul(
                                dn[:], lhsT=ones_sb[:], rhs=pT[:, kk, :],
                                start=(kk == 0), stop=(kk == nk - 1))
                            nc.tensor.matmul(
                                av[:], lhsT=v_sb[:, b * (T // 128) + kk,
                                               h * 128:(h + 1) * 128],
                                rhs=pT[:, kk, :], start=(kk == 0),
                                stop=(kk == nk - 1))

                        for kt in range(nk):
                            k0 = b * T + kt * KT
                            s_ps = sps.tile([128, QT], f32, name="s")
                            diag = kt * KT + KT - 1 > qt * QT
                            dd = kt * KT - qt * QT
                            nc.tensor.matmul(
                                s_ps[:], lhsT=kh[:, k0:k0 + KT],
                                rhs=qh[:, q0:q0 + QT], start=True,
                                stop=not diag)
                            if diag:   # -1e4 causal mask, added on the PE
                                nc.tensor.matmul(
                                    s_ps[:], lhsT=ident_sb[:],
                                    rhs=m0_sb[:, 384 - dd:384 - dd + QT],
                                    start=False, stop=True)
                            nc.scalar.activation(
                                pT[:, kt, :], s_ps[:], Exp, scale=cfg.SCALE)
                            if kt >= 3:   # dn/av lag 3 k-tiles behind scores
                                dn_av(kt - 3)
                        for kk in range(max(0, nk - 3), nk):
                            dn_av(kk)
                        dn_sb = atmp.tile([1, QT], f32, name="dn_sb")
                        rec = atmp.tile([1, QT], f32, name="rec")
                        rb = atmp.tile([128, QT], f32, name="rb")
                        y_sb = atmp.tile([128, QT], bf, name="y_sb")
                        nc.vector.tensor_copy(dn_sb[:], dn[:])
                        nc.vector.reciprocal_approx_fast(out=rec[:], in_=dn_sb[:])
                        rec_dr = rdram.tile([1, QT], f32, name="rec_dr")
                        nc.sync.dma_start(rec_dr[:], rec[:])
                        nc.sync.dma_start(
                            rb[:], rec_dr[:].partition_broadcast(128))
                        nc.vector.tensor_mul(y_sb[:], av[:], rb[:])
                        step = min(QT, SPL)
                        for s0 in range(0, QT, step):
                            g = q0 + s0
                            lo = g % RPC
                            nc.sync.dma_start(
                                a2a_in[h][lo // SPL][g // RPC, :,
                                                    lo % SPL:lo % SPL + step],
                                y_sb[:, s0:s0 + step])
                    # rows of (head h, slice s) complete -> exchange them now
                    nc.gpsimd.collective_compute(
                        "AllToAll", mybir.AluOpType.bypass,
                        replica_groups=[list(range(NC))],
                        ins=[a2a_in[h][s][:].opt()],
                        outs=[a2a_out[h][s][:].opt()])

        # ================= Phase 4: proj (per row-slice) =================
        with tc.tile_pool(name="proj_w2", bufs=1) as pw2pool, \
             tc.tile_pool(name="proj_y", bufs=2) as ypool, \
             tc.tile_pool(name="proj_o", bufs=2) as opool, \
             tc.tile_pool(name="proj_ps", bufs=4, space="PSUM") as pps:
            KH = KC // H_LOC            # contraction chunks per head-group
            for ch in range(1, C // CH):
                pw2 = pw2pool.tile([128, KC, CH], bf, name="pw2_sb")
                nc.sync.dma_start(
                    pw2[:], w_proj[:, ch * CH:(ch + 1) * CH]
                    .rearrange("(kc p) oc -> p kc oc", p=128))
                pw_halves.append(pw2)
            visits = [(ch, s) for s in range(NSPL) for ch in range(C // CH)]
            for (ch, s) in visits:
                pw = pw_halves[ch]
                if True:
                    for rt in range(SPL // 128):
                        yt = ypool.tile([128, KC, 128], bf, name="yt")
                        for h in range(H_LOC):
                            view = a2a_out[h][s].rearrange("sl ch r -> (sl ch) r")
                            nc.sync.dma_start(
                                yt[:, h * KH:(h + 1) * KH, :],
                                view[:, rt * 128:(rt + 1) * 128]
                                .rearrange("(kc p) r -> p kc r", p=128))
                        for ot in range(CH // OT):
                            oc0 = ch * CH + ot * OT
                            ps = pps.tile([128, OT], f32, name="o_ps")
                            for k in range(KC):
                                nc.tensor.matmul(
                                    ps[:], lhsT=yt[:, k, :],
                                    rhs=pw[:, k, ot * OT:(ot + 1) * OT],
                                    start=(k == 0), stop=(k == KC - 1))
                            o_sb = opool.tile([128, OT], f32, name="o_sb")
                            nc.vector.tensor_add(
                                o_sb[:], ps[:], bproj_sb[:, oc0:oc0 + OT])
                            r0 = s * SPL + rt * 128
                            nc.sync.dma_start(
                                out_ext[r0:r0 + 128, oc0:oc0 + OT],
                                o_sb[:])

    nc.compile()
    return nc


# ---------------------------------------------------------------- host prep


def host_prep(cfg: Cfg, x, W_attn, b_attn, lora_A_q, lora_B_q, lora_A_k,
              lora_B_k, W_proj, b_proj, lora_scaling=0.125):
    """Returns (in_maps, assemble_fn)."""
    B, T, C, NC, D = cfg.B, cfg.T, cfg.C, cfg.NC, cfg.D
    s = lora_scaling
    W = np.asarray(W_attn, FP32)
    bb = np.asarray(b_attn, FP32)
    Wq, Wk, Wv = W[:, :C], W[:, C:2 * C], W[:, 2 * C:]
    bq, bk, bv = bb[:C], bb[C:2 * C], bb[2 * C:]
    Aq = np.asarray(lora_A_q, FP32); Bq = np.asarray(lora_B_q, FP32)
    Ak = np.asarray(lora_A_k, FP32); Bk = np.asarray(lora_B_k, FP32)
    Wq_eff = Wq + (Wq @ Aq) @ Bq * s
    Wk_eff = Wk + (Wk @ Ak) @ Bk * s
    bq_eff = bq + (bq @ Aq) @ Bq * s
    bk_eff = bk + (bk @ Ak) @ Bk * s

    xT = np.ascontiguousarray(
        np.asarray(x, FP32).reshape(cfg.ROWS, C).T).astype(BF16)

    inv = 1.0 / (10000.0 ** (np.arange(0, D, 2, dtype=FP32) / D))
    tt = np.arange(T, dtype=FP32)
    fr = np.outer(tt, inv)
    cos = np.cos(np.concatenate([fr, fr], axis=1)).T.astype(BF16).copy()  # [128,T]
    sin = np.sin(np.concatenate([fr, fr], axis=1)).T.astype(FP32)
    sin[:64] *= -1.0
    sinTs = sin.astype(BF16).copy()

    kk = np.arange(128)[:, None]
    mm = np.arange(384 + cfg.QT)[None, :]
    M0 = np.where(mm >= kk + 384, 0.0, -1e4).astype(BF16)

    # permute W_proj rows to match the a2a_out channel order:
    # for each head-slot h: core 0's head h, core 1's head h, ...
    perm = np.concatenate(
        [np.arange(cfg.OCQ * i + 128 * h, cfg.OCQ * i + 128 * (h + 1))
         for h in range(cfg.H_LOC) for i in range(NC)])
    Wp = np.asarray(W_proj, FP32)[perm].astype(BF16)
    bp_rep = np.ascontiguousarray(
        np.broadcast_to(np.asarray(b_proj, FP32)[None, :], (128, C)))

    in_maps = []
    for c in range(NC):
        cs = slice(cfg.OCQ * c, cfg.OCQ * (c + 1))
        W_eff_c = np.concatenate(
            [Wq_eff[:, cs], Wk_eff[:, cs], Wv[:, cs]], axis=1).astype(BF16)
        bqk_c = np.concatenate([bq_eff[cs], bk_eff[cs]])          # [2*OCQ]
        bqk_c = np.ascontiguousarray(
            bqk_c.reshape(2 * cfg.H_LOC, 128).T).astype(FP32)     # [128, NQK]
        bv_c = np.ascontiguousarray(
            np.broadcast_to(bv[cs][None, :], (128, cfg.OCQ))).astype(FP32)
        in_maps.append({
            "xT": xT, "w_eff": W_eff_c, "b_qk": bqk_c, "b_v": bv_c,
            "w_proj": Wp, "b_proj": bp_rep, "cosT": cos, "sinTs": sinTs,
            "mask0": M0,
        })

    def assemble(results):
        out = np.concatenate([np.asarray(r["out"], FP32) for r in results], axis=0)
        return out.reshape(B, T, C)

    return in_maps, assemble


# ---------------------------------------------------------------- entry

_NC_CACHE = {}
LAST_RESULT = None


def kernel(x, W_attn, b_attn, lora_A_q, lora_B_q, lora_A_k, lora_B_k,
           W_proj, b_proj):
    global LAST_RESULT
    cfg = CFG
    if "full" not in _NC_CACHE:
        _NC_CACHE["full"] = build(cfg)
    nc = _NC_CACHE["full"]
    in_maps, assemble = host_prep(
        cfg, x, W_attn, b_attn, lora_A_q, lora_B_q, lora_A_k, lora_B_k,
        W_proj, b_proj)
    res = run_bass_kernel_spmd(nc, in_maps, core_ids=list(range(cfg.NC)))
    LAST_RESULT = res
    return assemble(res.results)


if __name__ == "__main__":
    cfg = Cfg(B=2, T=512, C=1024, H=8)
    nc = build(cfg, debug=True)
    print("build OK; instructions:",
          sum(len(b.instructions) for b in nc.main_func.blocks))
